# revision 16
# baseline (speedup 1.0000x reference)
"""Self-contained Trainium2 kernel for the per-sample channel-attention layer.

Reference computation (per batch sample, with q = x reshaped [c, h*w]):
    energy = q @ q.T                  # [c, c]
    attn   = softmax(energy, axis=-1)
    out    = attn @ q                 # [c, h*w]
    out    = w2 @ out + b             # 1x1 conv = channel mixing

Strategy: data-parallel over batch (b=8) across 8 NeuronCores — attention is
fully per-sample, so there is no cross-core communication at all. Per core:

  Phase A: load x with f32->bf16 cast-during-DMA (SWDGE); build q^T tiles with
           XBAR DMA transposes (SBUF->SBUF, no TensorE time); accumulate
           energy = q q^T in PSUM with k-subtile-batched matmuls (the XBAR
           row interleave n = p*16+k is fine: energy sums over all pixels,
           and lhsT/rhs take the same interleave from the same tile).
  Phase B: softmax over key channels (rows of [256, 256] energy) -> attn bf16.
           Fold the 1x1 conv into the attention matmul:
           w2 @ (attn @ q) == (w2 @ attn) @ q, so build M^T = attn^T-free
           matmul from attn (natural layout) and w2^T.
  Phase CD: final = M @ q + b streamed over pixels in 512-wide chunks, f32 out.

All matmuls run in bf16 with f32 PSUM accumulation (softmax logits are
dominated by the diagonal by ~16000, so bf16 energy is far more than enough;
the conv path sees ~0.3% relative error).
"""

import numpy as np

import concourse.bacc as bacc
import concourse.tile as tile
from concourse import mybir
from concourse.bass_utils import run_bass_kernel_spmd
from concourse.masks import make_identity

B, C, H, W = 8, 256, 128, 128
N = H * W            # 16384 pixels
NCORES = 8
ACH = 2048           # phase-A streaming chunk width (pixels)
NACH = N // ACH      # 8
NSUB = ACH // 128    # 16 k-subtiles per chunk
DCH = 512            # phase-CD output chunk width (one f32 PSUM bank)
NDCH = N // DCH      # 32

F32 = mybir.dt.float32
BF16 = mybir.dt.bfloat16
F8 = mybir.dt.float8e4
AX = mybir.AxisListType.X
AF = mybir.ActivationFunctionType
DR = mybir.MatmulPerfMode.DoubleRow

_CACHE = {}


def _build():
    nc = bacc.Bacc(None, target_bir_lowering=False, debug=False)
    x_ext = nc.dram_tensor("x", [C, N], F32, kind="ExternalInput")
    w_ext = nc.dram_tensor("conv_w", [C, C], F32, kind="ExternalInput")
    b_ext = nc.dram_tensor("conv_b", [C, 1], F32, kind="ExternalInput")
    out_ext = nc.dram_tensor("out", [C, N], F32, kind="ExternalOutput")

    with tile.TileContext(nc) as tc:
        with (
            tc.tile_pool(name="const", bufs=1) as const,
            tc.tile_pool(name="qpool", bufs=1) as qpool,
            tc.tile_pool(name="qtp", bufs=3) as qtp,
            tc.tile_pool(name="stage", bufs=2) as stage,
            tc.tile_pool(name="small", bufs=2) as small,
            tc.tile_pool(name="outp", bufs=3) as outp,
            tc.tile_pool(name="ps_t", bufs=2, space="PSUM") as ps_t,
            tc.tile_pool(name="ps_e", bufs=1, space="PSUM") as ps_e,
            tc.tile_pool(name="ps_cd", bufs=2, space="PSUM") as ps_cd,
        ):
            ident = const.tile([128, 128], BF16)
            make_identity(nc, ident)
            ident_f32 = const.tile([128, 128], F32)
            make_identity(nc, ident_f32)

            # conv weight: cast-load [o, c] to bf16, PE-transpose to
            # w2t_all[:, cb, :] = w2T[c_in rows, o] (block-contiguous c_in
            # subtiles, matching attn's i-block subtile split).
            wst = []
            for ob in range(2):
                wb = small.tile([128, C], BF16, tag=f"wb{ob}")
                nc.gpsimd.dma_start(out=wb, in_=w_ext[ob * 128:(ob + 1) * 128, :])
                wst.append(wb)
            w2t_all = const.tile([128, 2, C], BF16)
            for cb in range(2):
                wtp = ps_t.tile([128, 256], BF16, tag="tp")
                for ob in range(2):
                    nc.tensor.transpose(
                        wtp[:, ob * 128:(ob + 1) * 128],
                        wst[ob][:, cb * 128:(cb + 1) * 128],
                        ident,
                    )
                nc.vector.tensor_copy(out=w2t_all[:, cb, :], in_=wtp)

            bias = []
            for ob in range(2):
                bt = const.tile([128, 1], F32, tag=f"bias{ob}")
                nc.sync.dma_start(out=bt, in_=b_ext[ob * 128:(ob + 1) * 128, :])
                bias.append(bt)

            # ---- Phase A: cast-load x, XBAR-transpose, energy = q q^T ----
            e_ps = [
                ps_e.tile([128, C], F32, tag=f"e{ib}", name=f"e_ps{ib}")
                for ib in range(2)
            ]
            qtiles = []
            for ci in range(NACH):
                sl = slice(ci * ACH, (ci + 1) * ACH)
                # Loads go on the ACT-HWDGE and SWDGE queues; the SP-HWDGE
                # queue is reserved for the XBAR transposes (mixing transpose
                # and copy DMAs on one queue forces xbar-mode serialization).
                xf = stage.tile([128, 2, ACH], F32, tag="xf")
                for cb in range(2):
                    eng = nc.scalar if cb == 0 else nc.gpsimd
                    eng.dma_start(
                        out=xf[:, cb, :], in_=x_ext[cb * 128:(cb + 1) * 128, sl]
                    )
                qc = qpool.tile([128, 2, ACH], BF16, tag=f"q_{ci}", name=f"q_{ci}")
                nc.scalar.copy(out=qc[:, 0, :], in_=xf[:, 0, :])
                nc.vector.tensor_copy(out=qc[:, 1, :], in_=xf[:, 1, :])
                qtiles.append(qc)

                # qt[p, k, c] = q[c, ci*ACH + p*NSUB + k] via XBAR transpose
                qt = qtp.tile([128, NSUB, C], BF16, tag="qt")
                for cb in range(2):
                    nc.sync.dma_start(
                        out=qt[:, :, cb * 128:(cb + 1) * 128],
                        in_=qc[:, cb, :],
                        transpose=True,
                    )
                qt8 = qtp.tile([128, NSUB, C], F8, tag="qt8")
                half = NSUB // 2
                nc.vector.tensor_copy(out=qt8[:, :half, :], in_=qt[:, :half, :])
                nc.scalar.copy(out=qt8[:, half:, :], in_=qt[:, half:, :])

                # fp8 DoubleRow energy, upper-triangular blocks only:
                # E00|E01 from i0, E11 from i1; E10 is patched as E01^T below.
                for k2 in range(NSUB // 2):
                    pr = slice(2 * k2, 2 * k2 + 2)
                    first = ci == 0 and k2 == 0
                    last = ci == NACH - 1 and k2 == NSUB // 2 - 1
                    nc.tensor.matmul(
                        e_ps[0],
                        qt8[:, pr, 0:128],
                        qt8[:, pr, :],
                        start=first,
                        stop=last,
                        perf_mode=DR,
                        skip_group_check=True,
                    )
                    nc.tensor.matmul(
                        e_ps[1][:, 128:256],
                        qt8[:, pr, 128:256],
                        qt8[:, pr, 128:256],
                        start=first,
                        stop=last,
                        perf_mode=DR,
                        skip_group_check=True,
                    )

            # Patch E10 = E01^T (energy is symmetric).
            e01 = small.tile([128, 128], F32)
            nc.vector.tensor_copy(out=e01, in_=e_ps[0][:, 128:256])
            e10p = ps_t.tile([128, 128], F32, tag="tp")
            nc.tensor.transpose(e10p, e01, ident_f32)
            nc.vector.tensor_copy(out=e_ps[1][:, 0:128], in_=e10p)

            # ---- Phase B: softmax rows of energy -> attn (bf16, natural) ----
            at_all = small.tile([128, 2, C], BF16)
            for ib in range(2):
                nmx = small.tile([128, 1], F32, tag=f"nmx{ib}")
                nc.vector.reduce_max(out=nmx, in_=e_ps[ib], axis=AX, negate=True)
                pex = small.tile([128, C], BF16, tag=f"pex{ib}")
                ssum = small.tile([128, 1], F32, tag=f"ssum{ib}")
                nc.scalar.activation(
                    out=pex, in_=e_ps[ib], func=AF.Exp,
                    bias=nmx, scale=1.0, accum_out=ssum,
                )
                rec = small.tile([128, 1], F32, tag=f"rec{ib}")
                nc.vector.reciprocal(out=rec, in_=ssum)
                nc.vector.tensor_scalar_mul(out=at_all[:, ib, :], in0=pex,
                                            scalar1=rec)

            # M^T = (w2 @ attn)^T: lhsT = attn (natural [i, j]), rhs = w2T.
            mt_all = small.tile([128, 2, C], BF16)
            for jb in range(2):
                jsl = slice(jb * 128, (jb + 1) * 128)
                mtp = ps_t.tile([128, C], F32, tag="tp")
                for ib in range(2):
                    nc.tensor.matmul(mtp, at_all[:, ib, jsl], w2t_all[:, ib, :],
                                     start=ib == 0, stop=ib == 1)
                nc.vector.tensor_copy(out=mt_all[:, jb, :], in_=mtp)

            # ---- Phase CD: final = M @ q + b, streamed over pixels ----
            for cj in range(NDCH):
                ci, off = divmod(cj * DCH, ACH)
                qs = qtiles[ci][:, :, off:off + DCH]
                sl = slice(cj * DCH, (cj + 1) * DCH)
                fp = ps_cd.tile([128, 2, DCH], F32, tag="fp")
                for ob in range(2):
                    for cb in range(2):
                        nc.tensor.matmul(
                            fp[:, ob, :],
                            mt_all[:, cb, ob * 128:(ob + 1) * 128],
                            qs[:, cb, :],
                            start=cb == 0,
                            stop=cb == 1,
                        )
                st = [nc.gpsimd, nc.scalar, nc.sync]
                f0 = outp.tile([128, DCH], F32, tag="f0")
                nc.vector.tensor_scalar_add(out=f0, in0=fp[:, 0, :],
                                            scalar1=bias[0])
                st[(2 * cj) % 3].dma_start(out=out_ext[0:128, sl], in_=f0)
                f1 = outp.tile([128, DCH], F32, tag="f1")
                nc.scalar.add(out=f1, in_=fp[:, 1, :], add=bias[1])
                st[(2 * cj + 1) % 3].dma_start(out=out_ext[128:256, sl], in_=f1)

    nc.compile()
    return nc


def _get_nc():
    if "nc" not in _CACHE:
        _CACHE["nc"] = _build()
    return _CACHE["nc"]


def kernel(x, conv_w, conv_b):
    x = np.ascontiguousarray(np.asarray(x), dtype=np.float32)
    w2 = np.ascontiguousarray(np.asarray(conv_w, dtype=np.float32)[:, :, 0, 0])
    bb = np.ascontiguousarray(np.asarray(conv_b, dtype=np.float32).reshape(C, 1))
    nc = _get_nc()
    in_maps = [
        {"x": np.ascontiguousarray(x[i].reshape(C, N)), "conv_w": w2, "conv_b": bb}
        for i in range(B)
    ]
    res = run_bass_kernel_spmd(nc, in_maps, core_ids=list(range(NCORES)))
    out = np.stack(
        [res.results[i]["out"].reshape(C, H, W) for i in range(B)], axis=0
    )
    return out
